# revision 11
# baseline (speedup 1.0000x reference)
"""Trainium2 Bass kernel for GNN message passing (nn_FALR2_35794257445089).

Math (per batch element b, per-core shapes):
    z = concat(node_fts, hidden)                       (n, 2h)
    msgs[i, j, m] = msg1[j,m] + msg2[i,m] + msgE[i,j,m] + msgG[m]
    out_msgs[j, m] = max_i msgs[i,j,m] * adj[i,j]
    ret = z @ W_o1 + b_o1 + out_msgs @ W_o2 + b_o2

Strategy: data-parallel over b across 8 cores; each core streams its own
32 MiB edge_fts[b] slice once (memory-bound regime).

The platform's effective DMA bandwidth (~25-30 GB/s per core, measured)
makes this kernel DMA-byte-bound, so edge_fts ships as bf16 (16 MiB
instead of 32 MiB per core); everything precision-sensitive stays f32.

Reformulation (exact, f32 on device):
- Additive masking: with M'[i,j] = (adj[i,j]-1)*1e9 the candidates
  become msgE + c + M' (c = msg2 + msgG + biases), the i-independent
  msg1 pulls out of the max, and the reference's "masked entries
  contribute 0" / "all-masked columns give 0" semantics are restored by
  a final per-column clamp max(., zb[j]).
- Per-plane accumulation in PSUM via three chained matmuls:
  W_me^T @ edgeT (bf16 inputs, f32 accumulate) + identity-add of cT
  (f32) + rank-1 ones (x) M'[.,j] (f32, packed at 32-aligned partition
  groups to satisfy the LDW base-partition constraint).

Device inner loop (j = target node, 256 iters, "j-outer"):
  DMA : contiguous 1 MiB bf16 loads (128 i-partitions x 8 KiB lines)
  PE  : 2 transposes of edge[:,j,:] (i,h)->(h,i) + 3 accumulating
        matmuls (N=256) into a PSUM plane (m, i)
  ACT : copy transposed-edge PSUM->SBUF (matmul input)
  DVE : one grouped tensor_reduce(max) over 6 planes along i,
        writing acc[:, j0:j0+6] columns directly (no merge step)
Epilogue: A = acc + msg1T; transpose, clamp vs zb, transpose back;
ret = msgsT^T @ W_o2 + (z @ W_o1 + biases) via identity-add matmul.

Hardware-constraint workarounds:
- walrus allows only ONE embedded sync-wait per instruction; Bacc's
  generate_event_semaphores splits the rest, but to keep the hot loop
  free of split-sem overhead the kernel (a) pre-covers all constant-DMA
  semaphores on the PE clock with tiny warmup transposes, and (b)
  routes the PSUM-slot WAR dependency (DVE reduce -> next group's first
  matmul) through the ACT copy via an explicit tile dep.
"""

import os
import sys

import numpy as np

if "/opt/trn_rl_repo" not in sys.path:
    sys.path.insert(0, "/opt/trn_rl_repo")

import concourse.bass as bass
import concourse.bacc as bacc
import concourse.mybir as mybir
import concourse.tile as tile
from concourse.tile import add_dep_helper
from concourse.bass_utils import run_bass_kernel_spmd

B, N, H, MID, OUT = 8, 256, 128, 128, 128
F32 = mybir.dt.float32
BF16 = mybir.dt.bfloat16
BIG = 1.0e9
NEG = -1.0e30

GROUPS = [6] * 42 + [4]  # 256 target nodes j
assert sum(GROUPS) == N


def build_nc():
    nc = bacc.Bacc("TRN2", target_bir_lowering=False, debug=False)

    edge = nc.dram_tensor("edge", [N, N, H], BF16, kind="ExternalInput")
    ct_d = nc.dram_tensor("ct", [MID, N], F32, kind="ExternalInput")
    mt_d = nc.dram_tensor("mt", [4, 64 * N], F32, kind="ExternalInput")
    ones_d = nc.dram_tensor("ones4", [4, 128], F32, kind="ExternalInput")
    wme16_d = nc.dram_tensor("wme16", [H, MID], BF16, kind="ExternalInput")
    id16_d = nc.dram_tensor("id16", [128, 128], BF16, kind="ExternalInput")
    msg1t_d = nc.dram_tensor("msg1t", [MID, N], F32, kind="ExternalInput")
    zwo1_d = nc.dram_tensor("zwo1", [N, OUT], F32, kind="ExternalInput")
    zbc_d = nc.dram_tensor("zbc", [128, 2], F32, kind="ExternalInput")
    wme_d = nc.dram_tensor("wme", [H, MID], F32, kind="ExternalInput")
    wo2_d = nc.dram_tensor("wo2", [MID, OUT], F32, kind="ExternalInput")
    ident_d = nc.dram_tensor("ident", [128, 128], F32, kind="ExternalInput")
    out_d = nc.dram_tensor("out", [N, OUT], F32, kind="ExternalOutput")

    with tile.TileContext(nc) as tc:
        with (
            tc.tile_pool(name="const", bufs=1) as cpool,
            tc.tile_pool(name="eraw", bufs=6) as rpool,
            tc.tile_pool(name="edt", bufs=6) as epool,
            tc.tile_pool(name="xt", bufs=2, space="PSUM") as xpool,
            tc.tile_pool(name="grp", bufs=2, space="PSUM") as gpool,
        ):
            # ---- constants ----
            ident_sb = cpool.tile([128, 128], F32)
            nc.sync.dma_start(out=ident_sb, in_=ident_d[:, :])
            wme_sb = cpool.tile([H, MID], F32)
            nc.sync.dma_start(out=wme_sb, in_=wme_d[:, :])
            wo2_sb = cpool.tile([MID, OUT], F32)
            nc.sync.dma_start(out=wo2_sb, in_=wo2_d[:, :])
            msg1t_sb = cpool.tile([MID, N], F32)
            nc.sync.dma_start(out=msg1t_sb, in_=msg1t_d[:, :])
            zwo1_sb = cpool.tile([128, 2, OUT], F32)
            nc.sync.dma_start(
                out=zwo1_sb, in_=zwo1_d.rearrange("(t p) m -> p t m", p=128)
            )
            zbc_sb = cpool.tile([128, 2], F32)
            nc.sync.dma_start(out=zbc_sb, in_=zbc_d[:, :])
            ct_sb = cpool.tile([MID, N], F32)
            nc.sync.dma_start(out=ct_sb, in_=ct_d[:, :])
            mt_sb = cpool.tile([128, 64 * N], F32)
            nc.sync.dma_start(out=mt_sb[0:128:32, :], in_=mt_d[:, :])
            ones_sb = cpool.tile([128, 128], F32)
            nc.sync.dma_start(out=ones_sb[0:128:32, :], in_=ones_d[:, :])
            wme16_sb = cpool.tile([H, MID], BF16)
            nc.sync.dma_start(out=wme16_sb, in_=wme16_d[:, :])
            id16_sb = cpool.tile([128, 128], BF16)
            nc.sync.dma_start(out=id16_sb, in_=id16_d[:, :])
            acc_sb = cpool.tile([MID, N], F32)

            # ---- PE warmup: cover every constant-DMA semaphore on the PE
            # clock so in-loop matmuls never need a second embedded wait.
            scratch = xpool.tile([128, 256], F32, name="scratch", tag="xt")
            warm_srcs = [
                (ident_sb, 128),
                (wme_sb, 128),
                (wo2_sb, 128),
                (msg1t_sb[:, 0:128], 128),
                (zwo1_sb[:, 0, :], 128),
                (zbc_sb, 128),
                (ct_sb[:, 0:128], 128),
                (mt_sb[0:1, 0:128], 1),
                (ones_sb[0:1, 0:128], 1),
            ]
            last_warm = None
            for src, k in warm_srcs:
                m = src.free_size()
                last_warm = nc.tensor.transpose(
                    out=scratch[0:m, 0:k], in_=src, identity=ident_sb[0:k, 0:k]
                )
            scr16 = xpool.tile([128, 256], BF16, name="scr16", tag="xt")
            for src in [id16_sb, wme16_sb]:
                last_warm = nc.tensor.transpose(
                    out=scr16[0:128, 0:128], in_=src, identity=id16_sb
                )

            # ---- main loop over target nodes j ----
            raw_tiles = {}

            def get_raw(jc, ih):
                key = (jc, ih)
                if key not in raw_tiles:
                    rt = rpool.tile(
                        [128, 32, H], BF16, name=f"raw{jc}_{ih}", tag="raw"
                    )
                    nc.sync.dma_start(
                        out=rt,
                        in_=edge[
                            ih * 128 : (ih + 1) * 128, jc * 32 : (jc + 1) * 32, :
                        ],
                    )
                    raw_tiles[key] = rt
                return raw_tiles[key]

            reduce_insts = []
            j0 = 0
            for gi, gsz in enumerate(GROUPS):
                grp = gpool.tile([128, 1536], F32, name=f"grp{gi}", tag="grp")
                for dj in range(gsz):
                    j = j0 + dj
                    jc, jl = j // 32, j % 32
                    r0 = get_raw(jc, 0)
                    r1 = get_raw(jc, 1)
                    xt = xpool.tile([128, 256], BF16, name=f"xt{j}", tag="xt")
                    t1 = nc.tensor.transpose(
                        out=xt[:, 0:128], in_=r0[:, jl, :], identity=id16_sb
                    )
                    if j == 0:
                        add_dep_helper(
                            t1.ins, last_warm.ins, reason="pe warmup first"
                        )
                    nc.tensor.transpose(
                        out=xt[:, 128:256], in_=r1[:, jl, :], identity=id16_sb
                    )
                    edt = epool.tile([128, 256], BF16, name=f"edt{j}", tag="edt")
                    cp = nc.scalar.copy(out=edt, in_=xt[:, 0:256])
                    if reduce_insts:
                        # route the grp-slot WAR dependency through ACT so the
                        # matmul below needs only its single ACT wait.
                        add_dep_helper(
                            cp.ins,
                            reduce_insts[-1].ins,
                            reason="psum slot via act",
                        )
                    pl = grp[:, dj * 256 : (dj + 1) * 256]
                    nc.tensor.matmul(
                        out=pl, lhsT=wme16_sb, rhs=edt, start=True, stop=False
                    )
                    nc.tensor.matmul(
                        out=pl, lhsT=ident_sb, rhs=ct_sb, start=False, stop=False
                    )
                    g4 = 32 * (j % 4)
                    nc.tensor.matmul(
                        out=pl,
                        lhsT=ones_sb[g4 : g4 + 1, 0:128],
                        rhs=mt_sb[g4 : g4 + 1, (j // 4) * 256 : (j // 4 + 1) * 256],
                        start=False,
                        stop=True,
                        tile_position=(g4, 0),
                    )
                rin = grp[:, 0 : gsz * 256].rearrange("p (c i) -> p c i", i=256)
                reduce_insts.append(
                    nc.vector.tensor_reduce(
                        out=acc_sb[:, j0 : j0 + gsz],
                        in_=rin,
                        axis=mybir.AxisListType.X,
                        op=mybir.AluOpType.max,
                    )
                )
                j0 += gsz

            # ---- epilogue ----
            a_sb = cpool.tile([MID, N], F32)
            nc.vector.tensor_tensor(
                out=a_sb, in0=acc_sb, in1=msg1t_sb, op=mybir.AluOpType.add
            )
            xtf = xpool.tile([128, 256], F32, name="xtf", tag="xt")
            nc.tensor.transpose(
                out=xtf[:, 0:128], in_=a_sb[:, 0:128], identity=ident_sb
            )
            nc.tensor.transpose(
                out=xtf[:, 128:256], in_=a_sb[:, 128:256], identity=ident_sb
            )
            msgs_sb = cpool.tile([128, 2, MID], F32)
            nc.vector.tensor_scalar(
                out=msgs_sb[:, 0, :],
                in0=xtf[:, 0:128],
                scalar1=zbc_sb[:, 0:1],
                scalar2=None,
                op0=mybir.AluOpType.max,
            )
            nc.vector.tensor_scalar(
                out=msgs_sb[:, 1, :],
                in0=xtf[:, 128:256],
                scalar1=zbc_sb[:, 1:2],
                scalar2=None,
                op0=mybir.AluOpType.max,
            )
            xtg = xpool.tile([128, 256], F32, name="xtg", tag="xt")
            nc.tensor.transpose(
                out=xtg[:, 0:128], in_=msgs_sb[:, 0, :], identity=ident_sb
            )
            nc.tensor.transpose(
                out=xtg[:, 128:256], in_=msgs_sb[:, 1, :], identity=ident_sb
            )
            msgst_sb = cpool.tile([MID, N], F32)
            nc.scalar.copy(out=msgst_sb, in_=xtg[:, 0:256])
            out_ps = xpool.tile([128, 256], F32, name="out_ps", tag="xt")
            for t in range(2):
                sl = out_ps[:, t * 128 : (t + 1) * 128]
                nc.tensor.matmul(
                    out=sl,
                    lhsT=msgst_sb[:, t * 128 : (t + 1) * 128],
                    rhs=wo2_sb,
                    start=True,
                    stop=False,
                )
                nc.tensor.matmul(
                    out=sl,
                    lhsT=ident_sb,
                    rhs=zwo1_sb[:, t, :],
                    start=False,
                    stop=True,
                )
            out_sb = cpool.tile([128, 2, OUT], F32)
            nc.scalar.copy(out=out_sb, in_=out_ps[:, 0:256])
            nc.sync.dma_start(
                out=out_d.rearrange("(t p) m -> p t m", p=128), in_=out_sb
            )
    nc.compile()
    return nc


_NC_CACHE = {}


def _get_nc():
    if "nc" not in _NC_CACHE:
        _NC_CACHE["nc"] = build_nc()
    return _NC_CACHE["nc"]


def prepare_inputs(
    node_fts, edge_fts, graph_fts, adj_mat, hidden,
    W_m1, b_m1, W_m2, b_m2, W_me, b_me, W_mg, b_mg, W_o1, b_o1, W_o2, b_o2,
):
    f32 = np.float32
    z = np.concatenate([node_fts, hidden], axis=-1).astype(f32)  # (B, N, 2H)
    msg1t = (z @ W_m1 + b_m1).transpose(0, 2, 1)  # (B, MID, N)
    cvec = graph_fts @ W_mg + (b_m2 + b_me + b_mg)  # (B, MID)
    c = z @ W_m2 + cvec[:, None, :]  # (B, N, MID) indexed by source i

    import ml_dtypes

    edge16 = np.asarray(edge_fts, dtype=f32).astype(ml_dtypes.bfloat16)
    ct = np.ascontiguousarray(c.transpose(0, 2, 1), dtype=f32)  # (B, MID, N)
    mprime = ((adj_mat.astype(f32) - 1.0) * BIG).astype(f32)  # (B, N, N)
    # mt: rank-1 mask rows, row j packed at partition-group j%4, cols (j//4)*N
    mpT = mprime.transpose(0, 2, 1)  # (B, j, i)
    mt = (
        mpT.reshape(B, 64, 4, N).transpose(0, 2, 1, 3).reshape(B, 4, 64 * N)
    )
    ones4 = np.ones((4, 128), f32)

    anyzero = adj_mat.min(axis=1) == 0  # (B, N) per target column j
    zb = np.where(anyzero, 0.0, NEG).astype(f32)
    zbc = zb.reshape(B, 2, 128).transpose(0, 2, 1)  # (B, 128, 2)
    zwo1 = z @ W_o1 + (b_o1 + b_o2)  # (B, N, OUT)

    ident = np.eye(128, dtype=f32)
    import ml_dtypes  # noqa: F811
    in_maps = []
    for b in range(B):
        in_maps.append(
            {
                "edge": np.ascontiguousarray(edge16[b]),
                "ct": ct[b],
                "mt": np.ascontiguousarray(mt[b]),
                "ones4": ones4,
                "msg1t": np.ascontiguousarray(msg1t[b], dtype=f32),
                "zwo1": np.ascontiguousarray(zwo1[b], dtype=f32),
                "zbc": np.ascontiguousarray(zbc[b], dtype=f32),
                "wme": np.asarray(W_me, dtype=f32),
                "wme16": np.asarray(W_me, dtype=f32).astype(ml_dtypes.bfloat16),
                "wo2": np.asarray(W_o2, dtype=f32),
                "ident": ident,
                "id16": ident.astype(ml_dtypes.bfloat16),
            }
        )
    return in_maps


def kernel(**inputs):
    inputs = {k: np.asarray(v) for k, v in inputs.items()}
    in_maps = prepare_inputs(**inputs)
    nc = _get_nc()
    res = run_bass_kernel_spmd(nc, in_maps, list(range(B)))
    return np.stack([np.asarray(res.results[b]["out"]) for b in range(B)])


if __name__ == "__main__":
    print("smoke build only")
    build_nc()
    print("build ok")
